# revision 1
# baseline (speedup 1.0000x reference)
"""Trainium2 Bass kernel for nn_CrossTransformerBlock (sparse kNN cross-attention).

Shapes (hardcoded): B=4, NQ=4096, N=2048, DIM=128, DG=256, DI=256, K=16.
Sharding: 8 cores = (batch b, query-half h); each core handles 2048 queries
against its batch's 2048 points.

Per-core pipeline:
  prep:  PE projects k_all/v_all (points @ W), g_all = W_d1@xyz, u = W_d1@xyz_q+b;
         PE-transposes them into a row-major DRAM table [2048, 384] f16 (k|v|g).
  topk:  PE computes s = 2 q.x - |x|^2 (float32r, K=4) -> PSUM [128q, 2048n];
         DVE ORs the column index into the mantissa low bits (s|iota), then
         8x InstMax (top-8 per 256-chunk) + match_replace merge -> top-16
         packed values; idx = value & 0x7FF.
  gather: a selector matmul (E[q',p]=1[q'%16==p%16]) rearranges per-query idx
         into the wrapped int16 layout dma_gather wants; dma_gather(transpose=True)
         pulls 16 neighbor rows/query into feature-major SBUF [128f, 3, 4096].
  mlp:   h=relu(u-g); pos=W_d2 h+b (ACT bias); g1pre=(pos+q_attn)-k;
         gamma MLP on PE with ACT Relu/Exp+bias on the PSUM evacuation;
         softmax denominator + weighted sum via contiguous-halves tree adds;
         global (K+1-th) slot folded in as per-batch constants.
"""

import numpy as np

import concourse.bass as bass
import concourse.bacc as bacc
import concourse.mybir as mybir
from concourse.tile import TileContext
from concourse.bass_utils import run_bass_kernel_spmd

F32 = mybir.dt.float32
F32R = mybir.dt.float32r
F16 = mybir.dt.float16
U32 = mybir.dt.uint32
I32 = mybir.dt.int32
I16 = mybir.dt.int16
ALU = mybir.AluOpType
ACTF = mybir.ActivationFunctionType

B, NQ, N, DIM, DG, DI, K = 4, 4096, 2048, 128, 256, 256, 16
NQC = 2048          # queries per core
QTILE = 128         # topk tile (queries on partitions)
NTILES = NQC // QTILE
QBLK = 256          # gather/MLP block
NBLK = NQC // QBLK
CHUNK = 128         # topk candidate chunk (top-8 per chunk)
NCHUNK = N // CHUNK
ROWF = 3 * DIM      # table row features (k|v|g)

_CACHE = {}


def _build(stage='full'):
    nc = bacc.Bacc("TRN2", target_bir_lowering=False, debug=False, num_devices=8)

    # ---- external inputs (per core) ----
    qx4 = nc.dram_tensor("qx4", [4, NQC], F32, kind="ExternalInput")
    xt4 = nc.dram_tensor("xt4", [4, N], F32, kind="ExternalInput")
    ptsT = nc.dram_tensor("ptsT", [128, 2 * N], F16, kind="ExternalInput")
    xyzq4 = nc.dram_tensor("xyzq4", [4, NQC], F16, kind="ExternalInput")
    xyzn4 = nc.dram_tensor("xyzn4", [4, N], F16, kind="ExternalInput")
    wk_l = nc.dram_tensor("wk_l", [128, 2 * DIM], F16, kind="ExternalInput")
    wv_l = nc.dram_tensor("wv_l", [128, 2 * DIM], F16, kind="ExternalInput")
    wd1_l = nc.dram_tensor("wd1_l", [4, DIM], F16, kind="ExternalInput")
    wd2_l = nc.dram_tensor("wd2_l", [DIM, DIM], F16, kind="ExternalInput")
    wg1_l = nc.dram_tensor("wg1_l", [DIM, DIM], F16, kind="ExternalInput")
    wg2_l = nc.dram_tensor("wg2_l", [DIM, DIM], F16, kind="ExternalInput")
    # per-partition column vectors [128, 1] f32
    colv = nc.dram_tensor("colv", [DIM, 6], F32, kind="ExternalInput")
    # constants
    esel = nc.dram_tensor("esel", [128, 128], F16, kind="ExternalInput")
    masks = nc.dram_tensor("masks", [128, 2 * 256], F16, kind="ExternalInput")
    ident16 = nc.dram_tensor("ident16", [128, 128], F16, kind="ExternalInput")
    ident32 = nc.dram_tensor("ident32", [128, 128], F32, kind="ExternalInput")

    out = nc.dram_tensor("out", [NQC, DIM], F32, kind="ExternalOutput")

    with TileContext(nc) as tc:
        with tc.tile_pool(name="const", bufs=1) as cpool, \
             tc.tile_pool(name="prep", bufs=1) as prep, \
             tc.tile_pool(name="work", bufs=2) as work, \
             tc.tile_pool(name="mlp", bufs=1) as mlp, \
             tc.tile_pool(name="psd", bufs=1, space="PSUM") as psd, \
             tc.tile_pool(name="psm", bufs=2, space="PSUM") as psm, \
             tc.tile_pool(name="pss", bufs=2, space="PSUM") as pss, \
             tc.tile_pool(name="dram", bufs=1, space="DRAM") as dpool:

            # ---------- load constants / operands ----------
            qx4_s = cpool.tile([4, NQC], F32)
            xt4_s = cpool.tile([4, N], F32)
            nc.sync.dma_start(qx4_s[:], qx4[:])
            nc.sync.dma_start(xt4_s[:], xt4[:])
            xyzq4_s = cpool.tile([4, NQC], F16)
            xyzn4_s = cpool.tile([4, N], F16)
            nc.sync.dma_start(xyzq4_s[:], xyzq4[:])
            nc.sync.dma_start(xyzn4_s[:], xyzn4[:])
            pts_s = cpool.tile([128, 2 * N], F16)
            nc.sync.dma_start(pts_s[:], ptsT[:])
            wk_s = cpool.tile([128, 2 * DIM], F16)
            wv_s = cpool.tile([128, 2 * DIM], F16)
            nc.sync.dma_start(wk_s[:], wk_l[:])
            nc.sync.dma_start(wv_s[:], wv_l[:])
            wd1_s = cpool.tile([4, DIM], F16)
            nc.sync.dma_start(wd1_s[:], wd1_l[:])
            wd2_s = cpool.tile([DIM, DIM], F16)
            wg1_s = cpool.tile([DIM, DIM], F16)
            wg2_s = cpool.tile([DIM, DIM], F16)
            nc.sync.dma_start(wd2_s[:], wd2_l[:])
            nc.sync.dma_start(wg1_s[:], wg1_l[:])
            nc.sync.dma_start(wg2_s[:], wg2_l[:])
            colv_s = cpool.tile([DIM, 6], F32)
            nc.sync.dma_start(colv_s[:], colv[:])
            b_d2 = colv_s[:, 0:1]
            b_g1 = colv_s[:, 1:2]
            b_g2 = colv_s[:, 2:3]
            qat = colv_s[:, 3:4]
            eg = colv_s[:, 4:5]
            egv = colv_s[:, 5:6]
            esel_s = cpool.tile([128, 128], F16)
            nc.sync.dma_start(esel_s[:], esel[:])
            masks_s = cpool.tile([128, 2 * 256], F16)
            nc.sync.dma_start(masks_s[:], masks[:])
            id16 = cpool.tile([128, 128], F16)
            id32 = cpool.tile([128, 128], F32)
            nc.sync.dma_start(id16[:], ident16[:])
            nc.sync.dma_start(id32[:], ident32[:])

            # chunk-local column index 0..255 repeated per 256-chunk
            iota = cpool.tile([128, N], I32)
            nc.gpsimd.iota(iota[:], pattern=[[0, NCHUNK], [1, CHUNK]],
                           base=0, channel_multiplier=0)
            # integer constants as per-partition columns (bitvec ALU ops
            # reject float immediates)
            bitc = cpool.tile([128, 4], U32)
            nc.vector.memset(bitc[:, 0:1], 0xFFFFFF80)
            nc.vector.memset(bitc[:, 1:2], 0x78)
            nc.vector.memset(bitc[:, 2:3], 4)
            nc.vector.memset(bitc[:, 3:4], 0x7F)

            # ---------- prep projections (feature-major) ----------
            kT = prep.tile([128, N], F16)
            vT = prep.tile([128, N], F16)
            gT = prep.tile([128, N], F16)
            uT = prep.tile([128, NQC], F16)
            for col in range(4):
                cs = slice(col * 512, (col + 1) * 512)
                acc_k = psm.tile([128, 512], F32, tag="mm")
                nc.tensor.matmul(acc_k[:], wk_s[:, 0:DIM],
                                 pts_s[:, col * 512:(col + 1) * 512],
                                 start=True, stop=False)
                nc.tensor.matmul(acc_k[:], wk_s[:, DIM:2 * DIM],
                                 pts_s[:, N + col * 512:N + (col + 1) * 512],
                                 start=False, stop=True)
                nc.scalar.copy(kT[:, cs], acc_k[:])
                acc_v = psm.tile([128, 512], F32, tag="mm")
                nc.tensor.matmul(acc_v[:], wv_s[:, 0:DIM],
                                 pts_s[:, col * 512:(col + 1) * 512],
                                 start=True, stop=False)
                nc.tensor.matmul(acc_v[:], wv_s[:, DIM:2 * DIM],
                                 pts_s[:, N + col * 512:N + (col + 1) * 512],
                                 start=False, stop=True)
                nc.scalar.copy(vT[:, cs], acc_v[:])
                acc_g = psm.tile([128, 512], F32, tag="mm")
                nc.tensor.matmul(acc_g[:], wd1_s[:], xyzn4_s[:, cs], start=True, stop=True)
                nc.scalar.copy(gT[:, cs], acc_g[:])
                acc_u = psm.tile([128, 512], F32, tag="mm")
                nc.tensor.matmul(acc_u[:], wd1_s[:], xyzq4_s[:, cs], start=True, stop=True)
                nc.scalar.copy(uT[:, cs], acc_u[:])

            # ---------- build DRAM gather table [N, 384] f16 ----------
            table = dpool.tile([N, ROWF], F16)
            for c in range(N // 128):
                rs = slice(c * 128, (c + 1) * 128)
                row_sb = work.tile([128, ROWF], F16, tag="rowsb")
                for j, src in enumerate((kT, vT, gT)):
                    pt = pss.tile([128, 128], F16, tag="small")
                    nc.tensor.transpose(pt[:], src[:, rs], id16[:])
                    nc.scalar.copy(row_sb[:, j * 128:(j + 1) * 128], pt[:])
                nc.sync.dma_start(table[rs, :], row_sb[:])

            if stage == 'prep':
                osb0 = work.tile([128, 128], F32, tag="osb")
                nc.vector.tensor_copy(osb0[:], kT[:, 0:128])
                for r in range(16):
                    nc.sync.dma_start(out[r * 128:(r + 1) * 128, :], osb0[:])
                stages_done = True
            # ---------- main loop: 2 topk tiles + 1 gather/MLP block ----------
            for gb in range(NBLK if stage != 'prep' else 0):
                wif_list = []
                for t2 in range(2):
                    t = gb * 2 + t2
                    qs = slice(t * QTILE, (t + 1) * QTILE)
                    sps = psd.tile([128, N], F32, tag="dist")
                    for col in range(4):
                        cs = slice(col * 512, (col + 1) * 512)
                        nc.tensor.matmul(sps[:, cs], qx4_s[:, qs], xt4_s[:, cs],
                                         start=True, stop=True)
                    spk = work.tile([128, N], U32, tag="spk")
                    nc.vector.scalar_tensor_tensor(
                        spk[:], sps[:].bitcast(U32), bitc[:, 0:1],
                        iota[:].bitcast(U32), ALU.bitwise_and, ALU.bitwise_or)
                    spkf = spk[:].bitcast(F32)
                    cand = work.tile([128, NCHUNK * 8], F32, tag="cand")
                    for c in range(NCHUNK):
                        nc.vector.max(cand[:, c * 8:(c + 1) * 8],
                                      spkf[:, c * CHUNK:(c + 1) * CHUNK])
                    winners = work.tile([128, 16], F32, tag="win")
                    pos = work.tile([128, 16], U32, tag="pos")
                    nc.vector.max(winners[:, 0:8], cand[:])
                    nc.vector.max_index(pos[:, 0:8], winners[:, 0:8], cand[:])
                    nc.vector.match_replace(cand[:], winners[:, 0:8], cand[:], -3e38)
                    nc.vector.max(winners[:, 8:16], cand[:])
                    nc.vector.max_index(pos[:, 8:16], winners[:, 8:16], cand[:])
                    # global idx = (packed & 0xFF) + (pos//8)*256
                    base = work.tile([128, 16], U32, tag="base")
                    nc.vector.tensor_scalar(base[:], pos[:], bitc[:, 1:2],
                                            bitc[:, 2:3], ALU.bitwise_and,
                                            ALU.logical_shift_left)
                    wid = work.tile([128, 16], U32, tag="wid")
                    nc.vector.tensor_scalar(wid[:], winners[:].bitcast(U32),
                                            bitc[:, 3:4], None, ALU.bitwise_and)
                    nc.vector.tensor_tensor(wid[:], wid[:], base[:], ALU.add)
                    wif = work.tile([128, 16], F16, tag="wif")
                    nc.vector.tensor_copy(wif[:], wid[:])
                    wif_list.append(wif)

                if stage == 'topk':
                    osb0 = work.tile([128, 128], F32, tag="osb")
                    nc.vector.tensor_copy(osb0[:, 0:16], wif_list[0][:])
                    nc.vector.tensor_copy(osb0[:, 16:32], wif_list[1][:])
                    nc.sync.dma_start(out[gb * 256:gb * 256 + 128, :], osb0[:])
                    nc.sync.dma_start(out[gb * 256 + 128:gb * 256 + 256, :], osb0[:])
                    continue
                # --- selector matmul -> wrapped int16 idx layout ---
                psel = pss.tile([128, 256], F32, tag="small")
                for t2 in range(2):
                    rhs = work.tile([128, 256], F16, tag="rhs")
                    nc.vector.tensor_tensor(
                        rhs[:].rearrange("p (a b) -> p a b", a=16),
                        wif_list[t2][:].unsqueeze(2).broadcast_to((128, 16, 16)),
                        masks_s[:, t2 * 256:(t2 + 1) * 256].rearrange("p (a b) -> p a b", a=16),
                        ALU.mult)
                    nc.tensor.matmul(psel[:], esel_s[:], rhs[:],
                                     start=(t2 == 0), stop=(t2 == 1))
                idxs = work.tile([128, 256], I16, tag="idxs")
                nc.scalar.copy(idxs[:], psel[:])

                if stage == 'sel':
                    osb0 = work.tile([128, 128], F32, tag="osb")
                    nc.scalar.copy(osb0[:], psel[:, 0:128])
                    nc.sync.dma_start(out[gb * 256:gb * 256 + 128, :], osb0[:])
                    nc.sync.dma_start(out[gb * 256 + 128:gb * 256 + 256, :], osb0[:])
                    continue
                # --- gather 16 rows per query, feature-major ---
                gath = work.tile([128, 3, 4096], F16, tag="gath")
                nc.gpsimd.dma_gather(gath[:], table[:], idxs[:],
                                     num_idxs=4096, num_idxs_reg=4096,
                                     elem_size=ROWF, transpose=True,
                                     single_packet=False)
                if stage == 'gather':
                    osb0 = work.tile([128, 128], F32, tag="osb")
                    nc.vector.tensor_copy(osb0[:], gath[:, 0, 0:128])
                    nc.sync.dma_start(out[gb * 256:gb * 256 + 128, :], osb0[:])
                    nc.sync.dma_start(out[gb * 256 + 128:gb * 256 + 256, :], osb0[:])
                    continue
                k3 = gath[:, 0, :].rearrange("p (a b) -> p a b", a=16)
                v3 = gath[:, 1, :].rearrange("p (a b) -> p a b", a=16)
                g3 = gath[:, 2, :].rearrange("p (a b) -> p a b", a=16)
                ub = uT[:, gb * QBLK:(gb + 1) * QBLK].unsqueeze(1) \
                    .broadcast_to((128, 16, QBLK))

                # --- pos MLP layer 1: h = relu(u - g) ---
                hpre = mlp.tile([128, 4096], F16, tag="ma")
                h3 = hpre[:].rearrange("p (a b) -> p a b", a=16)
                nc.vector.tensor_tensor(h3, ub, g3, ALU.subtract)
                nc.vector.tensor_scalar_max(hpre[:], hpre[:], 0.0)

                # --- pos = W_d2 @ h + b_d2 ---
                pos = mlp.tile([128, 4096], F16, tag="mb")
                for col in range(8):
                    cs = slice(col * 512, (col + 1) * 512)
                    pm = psm.tile([128, 512], F32, tag="mm")
                    nc.tensor.matmul(pm[:], wd2_s[:], hpre[:, cs], start=True, stop=True)
                    nc.scalar.add(pos[:, cs], pm[:], b_d2)
                pos3 = pos[:].rearrange("p (a b) -> p a b", a=16)

                # --- g1pre = (pos + q_attn) - k ---
                g1pre = mlp.tile([128, 4096], F16, tag="mc")
                nc.vector.scalar_tensor_tensor(
                    g1pre[:].rearrange("p (a b) -> p a b", a=16),
                    pos3, qat, k3, ALU.add, ALU.subtract)

                # --- g1 = relu(W_g1 @ g1pre + b_g1) ---
                g1 = mlp.tile([128, 4096], F16, tag="ma")
                for col in range(8):
                    cs = slice(col * 512, (col + 1) * 512)
                    pm = psm.tile([128, 512], F32, tag="mm")
                    nc.tensor.matmul(pm[:], wg1_s[:], g1pre[:, cs], start=True, stop=True)
                    nc.scalar.activation(g1[:, cs], pm[:], ACTF.Relu, bias=b_g1)

                # --- expt = exp(W_g2 @ g1 + b_g2) ---
                expt = mlp.tile([128, 4096], F16, tag="md")
                for col in range(8):
                    cs = slice(col * 512, (col + 1) * 512)
                    pm = psm.tile([128, 512], F32, tag="mm")
                    nc.tensor.matmul(pm[:], wg2_s[:], g1[:, cs], start=True, stop=True)
                    nc.scalar.activation(expt[:, cs], pm[:], ACTF.Exp, bias=b_g2)
                e3 = expt[:].rearrange("p (a b) -> p a b", a=16)

                # --- vpos = v + pos ---
                vpos = mlp.tile([128, 4096], F16, tag="mc")
                vp3 = vpos[:].rearrange("p (a b) -> p a b", a=16)
                nc.vector.tensor_tensor(vp3, v3, pos3, ALU.add)

                # --- esum tree (f16 halves, f32 final) ---
                e8 = mlp.tile([128, 8, QBLK], F16, tag="t8")
                nc.vector.tensor_tensor(e8[:], e3[:, 0:8, :], e3[:, 8:16, :], ALU.add)
                e4 = mlp.tile([128, 4, QBLK], F16, tag="t4")
                nc.vector.tensor_tensor(e4[:], e8[:, 0:4, :], e8[:, 4:8, :], ALU.add)
                e2 = mlp.tile([128, 2, QBLK], F16, tag="t2")
                nc.vector.tensor_tensor(e2[:], e4[:, 0:2, :], e4[:, 2:4, :], ALU.add)
                esum = mlp.tile([128, QBLK], F32, tag="es")
                nc.vector.tensor_tensor(esum[:], e2[:, 0, :], e2[:, 1, :], ALU.add)

                # --- wprod = expt * vpos ; wsum tree ---
                wp = mlp.tile([128, 4096], F16, tag="ma")
                wp3 = wp[:].rearrange("p (a b) -> p a b", a=16)
                nc.vector.tensor_tensor(wp3, e3, vp3, ALU.mult)
                w8 = mlp.tile([128, 8, QBLK], F16, tag="t8")
                nc.vector.tensor_tensor(w8[:], wp3[:, 0:8, :], wp3[:, 8:16, :], ALU.add)
                w4 = mlp.tile([128, 4, QBLK], F16, tag="t4")
                nc.vector.tensor_tensor(w4[:], w8[:, 0:4, :], w8[:, 4:8, :], ALU.add)
                w2 = mlp.tile([128, 2, QBLK], F16, tag="t2")
                nc.vector.tensor_tensor(w2[:], w4[:, 0:2, :], w4[:, 2:4, :], ALU.add)
                wsum = mlp.tile([128, QBLK], F32, tag="ws")
                nc.vector.tensor_tensor(wsum[:], w2[:, 0, :], w2[:, 1, :], ALU.add)

                # --- fold global slot; normalize ---
                nc.vector.tensor_scalar_add(esum[:], esum[:], eg)
                nc.vector.reciprocal(esum[:], esum[:])
                res = mlp.tile([128, QBLK], F32, tag="res")
                nc.vector.scalar_tensor_tensor(res[:], wsum[:], egv, esum[:],
                                               ALU.add, ALU.mult)

                # --- transpose out and store ---
                for t2 in range(2):
                    po = pss.tile([128, 128], F32, tag="small")
                    nc.tensor.transpose(po[:], res[:, t2 * 128:(t2 + 1) * 128], id32[:])
                    osb = work.tile([128, 128], F32, tag="osb")
                    nc.scalar.copy(osb[:], po[:])
                    nc.sync.dma_start(
                        out[gb * QBLK + t2 * 128: gb * QBLK + (t2 + 1) * 128, :],
                        osb[:])

    nc.compile()
    return nc


def _host_prep(inputs):
    """Build the 8 per-core input maps from full inputs (layout prep only)."""
    xyz_q = np.asarray(inputs["xyz_q"], np.float32)
    lat_rep = np.asarray(inputs["lat_rep"], np.float32)
    xyz = np.asarray(inputs["xyz"], np.float32)
    points = np.asarray(inputs["points"], np.float32)
    W_d1 = np.asarray(inputs["W_d1"], np.float32); b_d1 = np.asarray(inputs["b_d1"], np.float32)
    W_d2 = np.asarray(inputs["W_d2"], np.float32); b_d2 = np.asarray(inputs["b_d2"], np.float32)
    W_g1 = np.asarray(inputs["W_g1"], np.float32); b_g1 = np.asarray(inputs["b_g1"], np.float32)
    W_g2 = np.asarray(inputs["W_g2"], np.float32); b_g2 = np.asarray(inputs["b_g2"], np.float32)
    W_kg = np.asarray(inputs["W_kg"], np.float32)
    W_vg = np.asarray(inputs["W_vg"], np.float32)
    W_q = np.asarray(inputs["W_q"], np.float32)
    W_k = np.asarray(inputs["W_k"], np.float32)
    W_v = np.asarray(inputs["W_v"], np.float32)

    # per-batch global-slot constants
    q_attn = lat_rep @ W_q.T                      # [B, DIM]
    k_g = lat_rep @ W_kg.T
    v_g = lat_rep @ W_vg.T
    tg = q_attn - k_g
    g1g = np.maximum(tg @ W_g1.T + b_g1, 0.0)
    logit_g = g1g @ W_g2.T + b_g2
    exp_g = np.exp(logit_g)                       # [B, DIM]
    egv = exp_g * v_g

    # constants
    qp = np.arange(128)
    esel = (qp[:, None] % 16 == qp[None, :] % 16).astype(np.float16)  # [q',p]
    masks = np.zeros((2, 128, 256), np.float16)
    g_of = qp // 16                               # q' // 16 in 0..7
    for t in range(2):
        for nb in range(16):
            for g in range(16):
                masks[t, :, nb * 16 + g] = (g_of == (g - t * 8)).astype(np.float16)
    ident16 = np.eye(128, dtype=np.float16)
    ident32 = np.eye(128, dtype=np.float32)

    wd1_l = np.concatenate([W_d1.T, b_d1[None, :]], axis=0).astype(np.float16)  # [4,128]

    maps = []
    for core in range(8):
        b, h = core // 2, core % 2
        qsl = slice(h * NQC, (h + 1) * NQC)
        xq = xyz_q[b, qsl]                        # [2048, 3]
        xn = xyz[b]                               # [2048, 3]
        qx4 = np.concatenate([2.0 * xq.T, np.ones((1, NQC), np.float32)], axis=0)
        xt4 = np.concatenate([xn.T, -np.sum(xn * xn, axis=1)[None, :]], axis=0)
        xyzq4 = np.concatenate([xq.T, np.ones((1, NQC), np.float32)], axis=0).astype(np.float16)
        xyzn4 = np.concatenate([xn.T, np.zeros((1, N), np.float32)], axis=0).astype(np.float16)
        pT = points[b].T.astype(np.float16)          # [256, N]
        ptsT = np.concatenate([pT[0:128], pT[128:256]], axis=1)  # [128, 2N]
        colv = np.stack([b_d2, b_g1, b_g2, q_attn[b], exp_g[b], egv[b]],
                        axis=1).astype(np.float32)
        maps.append({
            "qx4": np.ascontiguousarray(qx4, np.float32),
            "xt4": np.ascontiguousarray(xt4, np.float32),
            "ptsT": np.ascontiguousarray(ptsT),
            "xyzq4": np.ascontiguousarray(xyzq4),
            "xyzn4": np.ascontiguousarray(xyzn4),
            "wk_l": np.ascontiguousarray(np.concatenate(
                [W_k.T[0:128], W_k.T[128:256]], axis=1).astype(np.float16)),
            "wv_l": np.ascontiguousarray(np.concatenate(
                [W_v.T[0:128], W_v.T[128:256]], axis=1).astype(np.float16)),
            "wd1_l": np.ascontiguousarray(wd1_l),
            "wd2_l": np.ascontiguousarray(W_d2.T.astype(np.float16)),
            "wg1_l": np.ascontiguousarray(W_g1.T.astype(np.float16)),
            "wg2_l": np.ascontiguousarray(W_g2.T.astype(np.float16)),
            "colv": np.ascontiguousarray(colv),
            "esel": np.ascontiguousarray(esel),
            "masks": np.ascontiguousarray(
                np.concatenate([masks[0], masks[1]], axis=1)),
            "ident16": ident16,
            "ident32": ident32,
        })
    return maps


def kernel(**inputs):
    if "nc" not in _CACHE:
        _CACHE["nc"] = _build()
    nc = _CACHE["nc"]
    maps = _host_prep(inputs)
    res = run_bass_kernel_spmd(nc, maps, core_ids=list(range(8)))
    out = np.empty((B, NQ, DIM), np.float32)
    for core in range(8):
        b, h = core // 2, core % 2
        out[b, h * NQC:(h + 1) * NQC, :] = res.results[core]["out"]
    return out



# revision 3
# speedup vs baseline: 1.3777x; 1.3777x over previous
"""Trainium2 Bass kernel for nn_CrossTransformerBlock (sparse kNN cross-attention).

Shapes (hardcoded): B=4, NQ=4096, N=2048, DIM=128, DG=256, DI=256, K=16.
Sharding: 8 cores = (batch b, query-half h); each core handles 2048 queries
against its batch's 2048 points.

Per-core pipeline (software-pipelined over 8 blocks of 256 queries):
  prep:  PE projects k_all/v_all (points @ W), g_all = W_d1@xyz, u = W_d1@xyz_q+b;
         PE-transposes them into a row-major DRAM table [2048, 384] f16 (k|v|g).
  per iteration it: issue MLP-front(it-1) [h=relu(u-g), pos matmuls],
         then topk(it) [PE dist -> DVE pack/top-16], then selector matmul ->
         wrapped i16 idx layout -> dma_gather(it) on GpSimd, then
         MLP-back(it-1) [gamma MLP, softmax, weighted sum, store].
  The issue order keeps the gather-gating chain (topk -> sel -> idxs) early
  so the GpSimd gathers (32us each, the bottleneck) run back-to-back.
  q_attn is folded into b_g1' = b_g1 + W_g1 @ q_attn on the host; the
  global (K+1-th) slot is folded in as per-batch constants eg/egv.
"""

import numpy as np

import concourse.bass as bass
import concourse.bacc as bacc
import concourse.mybir as mybir
from concourse.tile import TileContext
from concourse.bass_utils import run_bass_kernel_spmd

F32 = mybir.dt.float32
F16 = mybir.dt.float16
U32 = mybir.dt.uint32
I32 = mybir.dt.int32
I16 = mybir.dt.int16
ALU = mybir.AluOpType
ACTF = mybir.ActivationFunctionType

B, NQ, N, DIM, DG, DI, K = 4, 4096, 2048, 128, 256, 256, 16
NQC = 2048          # queries per core
QTILE = 128         # topk tile (queries on partitions)
QBLK = 256          # gather/MLP block
NBLK = NQC // QBLK
CHUNK = 128         # topk candidate chunk (top-8 per chunk)
NCHUNK = N // CHUNK
ROWF = 3 * DIM      # table row features (k|v|g)

_CACHE = {}


def _build():
    nc = bacc.Bacc("TRN2", target_bir_lowering=False, debug=False, num_devices=8)

    # ---- external inputs (per core) ----
    qx4 = nc.dram_tensor("qx4", [4, NQC], F32, kind="ExternalInput")
    xt4 = nc.dram_tensor("xt4", [4, N], F32, kind="ExternalInput")
    ptsT = nc.dram_tensor("ptsT", [128, 2 * N], F16, kind="ExternalInput")
    xyzq4 = nc.dram_tensor("xyzq4", [4, NQC], F16, kind="ExternalInput")
    xyzn4 = nc.dram_tensor("xyzn4", [4, N], F16, kind="ExternalInput")
    wk_l = nc.dram_tensor("wk_l", [128, 2 * DIM], F16, kind="ExternalInput")
    wv_l = nc.dram_tensor("wv_l", [128, 2 * DIM], F16, kind="ExternalInput")
    wd1_l = nc.dram_tensor("wd1_l", [4, DIM], F16, kind="ExternalInput")
    wd2_l = nc.dram_tensor("wd2_l", [DIM, DIM], F16, kind="ExternalInput")
    wg1_l = nc.dram_tensor("wg1_l", [DIM, DIM], F16, kind="ExternalInput")
    wg2_l = nc.dram_tensor("wg2_l", [DIM, DIM], F16, kind="ExternalInput")
    # per-partition column vectors [128, 5] f32: b_d2, b_g1', b_g2, eg, egv
    colv = nc.dram_tensor("colv", [DIM, 5], F32, kind="ExternalInput")
    # constants
    esel = nc.dram_tensor("esel", [128, 128], F16, kind="ExternalInput")
    masks = nc.dram_tensor("masks", [128, 2 * 256], F16, kind="ExternalInput")
    ident16 = nc.dram_tensor("ident16", [128, 128], F16, kind="ExternalInput")
    ident32 = nc.dram_tensor("ident32", [128, 128], F32, kind="ExternalInput")

    out = nc.dram_tensor("out", [NQC, DIM], F32, kind="ExternalOutput")

    with TileContext(nc) as tc:
        with tc.tile_pool(name="const", bufs=1) as cpool, \
             tc.tile_pool(name="prep", bufs=1) as prep, \
             tc.tile_pool(name="work", bufs=2) as work, \
             tc.tile_pool(name="mlp", bufs=1) as mlp, \
             tc.tile_pool(name="psd", bufs=1, space="PSUM") as psd, \
             tc.tile_pool(name="psm", bufs=2, space="PSUM") as psm, \
             tc.tile_pool(name="pss", bufs=2, space="PSUM") as pss, \
             tc.tile_pool(name="dram", bufs=1, space="DRAM") as dpool:

            # ---------- load constants / operands ----------
            qx4_s = cpool.tile([4, NQC], F32)
            xt4_s = cpool.tile([4, N], F32)
            nc.sync.dma_start(qx4_s[:], qx4[:])
            nc.sync.dma_start(xt4_s[:], xt4[:])
            xyzq4_s = cpool.tile([4, NQC], F16)
            xyzn4_s = cpool.tile([4, N], F16)
            nc.sync.dma_start(xyzq4_s[:], xyzq4[:])
            nc.sync.dma_start(xyzn4_s[:], xyzn4[:])
            pts_s = cpool.tile([128, 2 * N], F16)
            nc.sync.dma_start(pts_s[:], ptsT[:])
            wk_s = cpool.tile([128, 2 * DIM], F16)
            wv_s = cpool.tile([128, 2 * DIM], F16)
            nc.sync.dma_start(wk_s[:], wk_l[:])
            nc.sync.dma_start(wv_s[:], wv_l[:])
            wd1_s = cpool.tile([4, DIM], F16)
            nc.sync.dma_start(wd1_s[:], wd1_l[:])
            wd2_s = cpool.tile([DIM, DIM], F16)
            wg1_s = cpool.tile([DIM, DIM], F16)
            wg2_s = cpool.tile([DIM, DIM], F16)
            nc.sync.dma_start(wd2_s[:], wd2_l[:])
            nc.sync.dma_start(wg1_s[:], wg1_l[:])
            nc.sync.dma_start(wg2_s[:], wg2_l[:])
            colv_s = cpool.tile([DIM, 5], F32)
            nc.sync.dma_start(colv_s[:], colv[:])
            b_d2 = colv_s[:, 0:1]
            b_g1 = colv_s[:, 1:2]
            b_g2 = colv_s[:, 2:3]
            eg = colv_s[:, 3:4]
            egv = colv_s[:, 4:5]
            esel_s = cpool.tile([128, 128], F16)
            nc.sync.dma_start(esel_s[:], esel[:])
            masks_s = cpool.tile([128, 2 * 256], F16)
            nc.sync.dma_start(masks_s[:], masks[:])
            id16 = cpool.tile([128, 128], F16)
            id32 = cpool.tile([128, 128], F32)
            nc.sync.dma_start(id16[:], ident16[:])
            nc.sync.dma_start(id32[:], ident32[:])

            # chunk-local column index 0..127 repeated per 128-chunk
            iota = cpool.tile([128, N], I32)
            nc.gpsimd.iota(iota[:], pattern=[[0, NCHUNK], [1, CHUNK]],
                           base=0, channel_multiplier=0)
            # mask constant for the score pack (STT needs an AP scalar)
            bitc = cpool.tile([128, 1], U32)
            nc.vector.memset(bitc[:, 0:1], 0xFFFFFF80)

            # ---------- DVE uop warmups (first use of an ALU pair is slow) ----
            wu = cpool.tile([128, 16], U32)
            wuf = cpool.tile([128, 16], F16)
            wuo = cpool.tile([128, 16], U32)
            nc.vector.memset(wu[:], 1)
            nc.vector.scalar_tensor_tensor(wuo[:], wu[:], bitc[:, 0:1], wu[:],
                                           ALU.bitwise_and, ALU.bitwise_or)
            nc.vector.tensor_scalar(wuo[:], wu[:], 0x78, 4,
                                    ALU.bitwise_and, ALU.logical_shift_left)
            nc.vector.tensor_scalar(wuo[:], wu[:], 0x7F, None, ALU.bitwise_and)
            nc.vector.tensor_tensor(wuo[:], wu[:], wu[:], ALU.add)
            nc.vector.tensor_copy(wuf[:], wu[:])
            wf2 = cpool.tile([128, 16], F16)
            nc.vector.memset(wf2[:], 1.0)
            nc.vector.tensor_tensor(wf2[:, 0:8], wf2[:, 0:8], wf2[:, 8:16],
                                    ALU.subtract)
            nc.vector.tensor_tensor(wf2[:, 0:8], wf2[:, 0:8], wf2[:, 8:16],
                                    ALU.mult)
            nc.vector.tensor_scalar_max(wf2[:], wf2[:], 0.0)
            wmx = cpool.tile([128, 16], F32)
            wmi = cpool.tile([128, 8], U32)
            nc.vector.memset(wmx[:], 0.0)
            nc.vector.max(wmx[:, 0:8], wmx[:])
            nc.vector.max_index(wmi[:], wmx[:, 0:8], wmx[:])
            nc.vector.match_replace(wmx[:], wmx[:, 0:8], wmx[:], -3e38)

            # ---------- prep projections (feature-major) ----------
            kT = prep.tile([128, N], F16)
            vT = prep.tile([128, N], F16)
            gT = prep.tile([128, N], F16)
            uT = prep.tile([128, NQC], F16)
            for col in range(4):
                cs = slice(col * 512, (col + 1) * 512)
                acc_k = psm.tile([128, 512], F32, tag="mm")
                nc.tensor.matmul(acc_k[:], wk_s[:, 0:DIM],
                                 pts_s[:, col * 512:(col + 1) * 512],
                                 start=True, stop=False)
                nc.tensor.matmul(acc_k[:], wk_s[:, DIM:2 * DIM],
                                 pts_s[:, N + col * 512:N + (col + 1) * 512],
                                 start=False, stop=True)
                nc.scalar.copy(kT[:, cs], acc_k[:])
                acc_v = psm.tile([128, 512], F32, tag="mm")
                nc.tensor.matmul(acc_v[:], wv_s[:, 0:DIM],
                                 pts_s[:, col * 512:(col + 1) * 512],
                                 start=True, stop=False)
                nc.tensor.matmul(acc_v[:], wv_s[:, DIM:2 * DIM],
                                 pts_s[:, N + col * 512:N + (col + 1) * 512],
                                 start=False, stop=True)
                nc.scalar.copy(vT[:, cs], acc_v[:])
                acc_g = psm.tile([128, 512], F32, tag="mm")
                nc.tensor.matmul(acc_g[:], wd1_s[:], xyzn4_s[:, cs], start=True, stop=True)
                nc.scalar.copy(gT[:, cs], acc_g[:])
                acc_u = psm.tile([128, 512], F32, tag="mm")
                nc.tensor.matmul(acc_u[:], wd1_s[:], xyzq4_s[:, cs], start=True, stop=True)
                nc.scalar.copy(uT[:, cs], acc_u[:])

            # ---------- build DRAM gather table [N, 384] f16 ----------
            table = dpool.tile([N, ROWF], F16)
            for c in range(N // 128):
                rs = slice(c * 128, (c + 1) * 128)
                row_sb = work.tile([128, ROWF], F16, tag="rowsb")
                for j, src in enumerate((kT, vT, gT)):
                    pt = pss.tile([128, 128], F16, tag="small")
                    nc.tensor.transpose(pt[:], src[:, rs], id16[:])
                    nc.scalar.copy(row_sb[:, j * 128:(j + 1) * 128], pt[:])
                nc.sync.dma_start(table[rs, :], row_sb[:])

            # ---------- software-pipelined main loop ----------
            gath_tiles = {}

            def issue_topk(gb):
                wif_list = []
                for t2 in range(2):
                    t = gb * 2 + t2
                    qs = slice(t * QTILE, (t + 1) * QTILE)
                    sps = psd.tile([128, N], F32, tag="dist")
                    for col in range(4):
                        cs = slice(col * 512, (col + 1) * 512)
                        nc.tensor.matmul(sps[:, cs], qx4_s[:, qs], xt4_s[:, cs],
                                         start=True, stop=True)
                    spk = work.tile([128, N], U32, tag="spk")
                    nc.vector.scalar_tensor_tensor(
                        spk[:], sps[:].bitcast(U32), bitc[:, 0:1],
                        iota[:].bitcast(U32), ALU.bitwise_and, ALU.bitwise_or)
                    spkf = spk[:].bitcast(F32)
                    cand = work.tile([128, NCHUNK * 8], F32, tag="cand")
                    for c in range(NCHUNK):
                        nc.vector.max(cand[:, c * 8:(c + 1) * 8],
                                      spkf[:, c * CHUNK:(c + 1) * CHUNK])
                    winners = work.tile([128, 16], F32, tag="win")
                    pos = work.tile([128, 16], U32, tag="pos")
                    nc.vector.max(winners[:, 0:8], cand[:])
                    nc.vector.max_index(pos[:, 0:8], winners[:, 0:8], cand[:])
                    nc.vector.match_replace(cand[:], winners[:, 0:8], cand[:], -3e38)
                    nc.vector.max(winners[:, 8:16], cand[:])
                    nc.vector.max_index(pos[:, 8:16], winners[:, 8:16], cand[:])
                    # global idx = (packed & 0x7F) + (pos//8)*128
                    base = work.tile([128, 16], U32, tag="base")
                    nc.vector.tensor_scalar(base[:], pos[:], 0x78, 4,
                                            ALU.bitwise_and,
                                            ALU.logical_shift_left)
                    wid = work.tile([128, 16], U32, tag="wid")
                    nc.vector.tensor_scalar(wid[:], winners[:].bitcast(U32),
                                            0x7F, None, ALU.bitwise_and)
                    nc.vector.tensor_tensor(wid[:], wid[:], base[:], ALU.add)
                    wif = work.tile([128, 16], F16, tag="wif")
                    nc.vector.tensor_copy(wif[:], wid[:])
                    wif_list.append(wif)
                return wif_list

            def issue_sel_gather(gb, wif_list):
                psel = pss.tile([128, 256], F32, tag="small")
                for t2 in range(2):
                    rhs = work.tile([128, 256], F16, tag="rhs")
                    nc.vector.tensor_tensor(
                        rhs[:].rearrange("p (a b) -> p a b", a=16),
                        wif_list[t2][:].unsqueeze(2).broadcast_to((128, 16, 16)),
                        masks_s[:, t2 * 256:(t2 + 1) * 256].rearrange("p (a b) -> p a b", a=16),
                        ALU.mult)
                    nc.tensor.matmul(psel[:], esel_s[:], rhs[:],
                                     start=(t2 == 0), stop=(t2 == 1))
                idxs = work.tile([128, 256], I16, tag="idxs")
                nc.scalar.copy(idxs[:], psel[:])
                gath = work.tile([128, 3, 4096], F16, tag="gath")
                nc.gpsimd.dma_gather(gath[:], table[:], idxs[:],
                                     num_idxs=4096, num_idxs_reg=4096,
                                     elem_size=ROWF, transpose=True,
                                     single_packet=False)
                gath_tiles[gb] = gath

            def issue_mlp_front(gb):
                gath = gath_tiles[gb]
                g3 = gath[:, 2, :].rearrange("p (a b) -> p a b", a=16)
                ub = uT[:, gb * QBLK:(gb + 1) * QBLK].unsqueeze(1) \
                    .broadcast_to((128, 16, QBLK))
                # --- pos MLP layer 1: h = relu(u - g) ---
                hpre = mlp.tile([128, 4096], F16, tag="ma")
                h3 = hpre[:].rearrange("p (a b) -> p a b", a=16)
                nc.vector.tensor_tensor(h3, ub, g3, ALU.subtract)
                nc.vector.tensor_scalar_max(hpre[:], hpre[:], 0.0)
                # --- pos = W_d2 @ h + b_d2 ---
                pos = mlp.tile([128, 4096], F16, tag="mb")
                for col in range(8):
                    cs = slice(col * 512, (col + 1) * 512)
                    pm = psm.tile([128, 512], F32, tag="mm")
                    nc.tensor.matmul(pm[:], wd2_s[:], hpre[:, cs], start=True, stop=True)
                    nc.scalar.add(pos[:, cs], pm[:], b_d2)
                return pos

            def issue_mlp_back(gb, pos):
                gath = gath_tiles.pop(gb)
                k3 = gath[:, 0, :].rearrange("p (a b) -> p a b", a=16)
                v3 = gath[:, 1, :].rearrange("p (a b) -> p a b", a=16)
                pos3 = pos[:].rearrange("p (a b) -> p a b", a=16)

                # --- g1pre = pos - k  (q_attn folded into b_g1') ---
                g1pre = mlp.tile([128, 4096], F16, tag="mc")
                nc.vector.tensor_tensor(
                    g1pre[:].rearrange("p (a b) -> p a b", a=16),
                    pos3, k3, ALU.subtract)

                # --- g1 = relu(W_g1 @ g1pre + b_g1') ---
                g1 = mlp.tile([128, 4096], F16, tag="ma")
                for col in range(8):
                    cs = slice(col * 512, (col + 1) * 512)
                    pm = psm.tile([128, 512], F32, tag="mm")
                    nc.tensor.matmul(pm[:], wg1_s[:], g1pre[:, cs], start=True, stop=True)
                    nc.scalar.activation(g1[:, cs], pm[:], ACTF.Relu, bias=b_g1)

                # --- expt = exp(W_g2 @ g1 + b_g2) ---
                expt = mlp.tile([128, 4096], F16, tag="md")
                for col in range(8):
                    cs = slice(col * 512, (col + 1) * 512)
                    pm = psm.tile([128, 512], F32, tag="mm")
                    nc.tensor.matmul(pm[:], wg2_s[:], g1[:, cs], start=True, stop=True)
                    nc.scalar.activation(expt[:, cs], pm[:], ACTF.Exp, bias=b_g2)
                e3 = expt[:].rearrange("p (a b) -> p a b", a=16)

                # --- vpos = v + pos ---
                vpos = mlp.tile([128, 4096], F16, tag="mc")
                vp3 = vpos[:].rearrange("p (a b) -> p a b", a=16)
                nc.vector.tensor_tensor(vp3, v3, pos3, ALU.add)

                # --- esum tree (f16 halves, f32 final) ---
                e8 = mlp.tile([128, 8, QBLK], F16, tag="t8")
                nc.vector.tensor_tensor(e8[:], e3[:, 0:8, :], e3[:, 8:16, :], ALU.add)
                e4 = mlp.tile([128, 4, QBLK], F16, tag="t4")
                nc.vector.tensor_tensor(e4[:], e8[:, 0:4, :], e8[:, 4:8, :], ALU.add)
                e2 = mlp.tile([128, 2, QBLK], F16, tag="t2")
                nc.vector.tensor_tensor(e2[:], e4[:, 0:2, :], e4[:, 2:4, :], ALU.add)
                esum = mlp.tile([128, QBLK], F32, tag="es")
                nc.vector.tensor_tensor(esum[:], e2[:, 0, :], e2[:, 1, :], ALU.add)
                # --- fold global slot; reciprocal on ACT ---
                nc.vector.tensor_scalar_add(esum[:], esum[:], eg)
                rec = mlp.tile([128, QBLK], F32, tag="rc")
                nc.vector.reciprocal(rec[:], esum[:])

                # --- wprod = expt * vpos ; wsum tree ---
                wp = mlp.tile([128, 4096], F16, tag="ma")
                wp3 = wp[:].rearrange("p (a b) -> p a b", a=16)
                nc.vector.tensor_tensor(wp3, e3, vp3, ALU.mult)
                w8 = mlp.tile([128, 8, QBLK], F16, tag="t8")
                nc.vector.tensor_tensor(w8[:], wp3[:, 0:8, :], wp3[:, 8:16, :], ALU.add)
                w4 = mlp.tile([128, 4, QBLK], F16, tag="t4")
                nc.vector.tensor_tensor(w4[:], w8[:, 0:4, :], w8[:, 4:8, :], ALU.add)
                w2 = mlp.tile([128, 2, QBLK], F16, tag="t2")
                nc.vector.tensor_tensor(w2[:], w4[:, 0:2, :], w4[:, 2:4, :], ALU.add)
                wsum = mlp.tile([128, QBLK], F32, tag="ws")
                nc.vector.tensor_tensor(wsum[:], w2[:, 0, :], w2[:, 1, :], ALU.add)

                # --- res = (wsum + egv) * (1 / esum) ---
                res = mlp.tile([128, QBLK], F32, tag="res")
                nc.vector.scalar_tensor_tensor(res[:], wsum[:], egv, rec[:],
                                               ALU.add, ALU.mult)

                # --- transpose out and store ---
                for t2 in range(2):
                    po = pss.tile([128, 128], F32, tag="small")
                    nc.tensor.transpose(po[:], res[:, t2 * 128:(t2 + 1) * 128], id32[:])
                    osb = work.tile([128, 128], F32, tag="osb")
                    nc.scalar.copy(osb[:], po[:])
                    nc.sync.dma_start(
                        out[gb * QBLK + t2 * 128: gb * QBLK + (t2 + 1) * 128, :],
                        osb[:])

            pos_prev = None
            for it in range(NBLK + 1):
                if it > 0:
                    pos_prev = issue_mlp_front(it - 1)
                if it < NBLK:
                    wif_list = issue_topk(it)
                    issue_sel_gather(it, wif_list)
                if it > 0:
                    issue_mlp_back(it - 1, pos_prev)

    nc.compile()
    return nc


def _host_prep(inputs):
    """Build the 8 per-core input maps from full inputs (layout prep only)."""
    xyz_q = np.asarray(inputs["xyz_q"], np.float32)
    lat_rep = np.asarray(inputs["lat_rep"], np.float32)
    xyz = np.asarray(inputs["xyz"], np.float32)
    points = np.asarray(inputs["points"], np.float32)
    W_d1 = np.asarray(inputs["W_d1"], np.float32); b_d1 = np.asarray(inputs["b_d1"], np.float32)
    W_d2 = np.asarray(inputs["W_d2"], np.float32); b_d2 = np.asarray(inputs["b_d2"], np.float32)
    W_g1 = np.asarray(inputs["W_g1"], np.float32); b_g1 = np.asarray(inputs["b_g1"], np.float32)
    W_g2 = np.asarray(inputs["W_g2"], np.float32); b_g2 = np.asarray(inputs["b_g2"], np.float32)
    W_kg = np.asarray(inputs["W_kg"], np.float32)
    W_vg = np.asarray(inputs["W_vg"], np.float32)
    W_q = np.asarray(inputs["W_q"], np.float32)
    W_k = np.asarray(inputs["W_k"], np.float32)
    W_v = np.asarray(inputs["W_v"], np.float32)

    # per-batch global-slot constants
    q_attn = lat_rep @ W_q.T                      # [B, DIM]
    k_g = lat_rep @ W_kg.T
    v_g = lat_rep @ W_vg.T
    tg = q_attn - k_g
    g1g = np.maximum(tg @ W_g1.T + b_g1, 0.0)
    logit_g = g1g @ W_g2.T + b_g2
    exp_g = np.exp(logit_g)                       # [B, DIM]
    egv = exp_g * v_g
    # fold q_attn into the g1 bias: g1 = relu(W_g1 @ (pos - k) + b_g1')
    b_g1p = b_g1[None, :] + q_attn @ W_g1.T       # [B, DIM]

    # constants
    qp = np.arange(128)
    esel = (qp[:, None] % 16 == qp[None, :] % 16).astype(np.float16)  # [q',p]
    masks = np.zeros((2, 128, 256), np.float16)
    g_of = qp // 16                               # q' // 16 in 0..7
    for t in range(2):
        for nb in range(16):
            for g in range(16):
                masks[t, :, nb * 16 + g] = (g_of == (g - t * 8)).astype(np.float16)
    ident16 = np.eye(128, dtype=np.float16)
    ident32 = np.eye(128, dtype=np.float32)

    wd1_l = np.concatenate([W_d1.T, b_d1[None, :]], axis=0).astype(np.float16)  # [4,128]

    maps = []
    for core in range(8):
        b, h = core // 2, core % 2
        qsl = slice(h * NQC, (h + 1) * NQC)
        xq = xyz_q[b, qsl]                        # [2048, 3]
        xn = xyz[b]                               # [2048, 3]
        qx4 = np.concatenate([2.0 * xq.T, np.ones((1, NQC), np.float32)], axis=0)
        xt4 = np.concatenate([xn.T, -np.sum(xn * xn, axis=1)[None, :]], axis=0)
        xyzq4 = np.concatenate([xq.T, np.ones((1, NQC), np.float32)], axis=0).astype(np.float16)
        xyzn4 = np.concatenate([xn.T, np.zeros((1, N), np.float32)], axis=0).astype(np.float16)
        pT = points[b].T.astype(np.float16)          # [256, N]
        ptsT = np.concatenate([pT[0:128], pT[128:256]], axis=1)  # [128, 2N]
        colv = np.stack([b_d2, b_g1p[b], b_g2, exp_g[b], egv[b]],
                        axis=1).astype(np.float32)
        maps.append({
            "qx4": np.ascontiguousarray(qx4, np.float32),
            "xt4": np.ascontiguousarray(xt4, np.float32),
            "ptsT": np.ascontiguousarray(ptsT),
            "xyzq4": np.ascontiguousarray(xyzq4),
            "xyzn4": np.ascontiguousarray(xyzn4),
            "wk_l": np.ascontiguousarray(np.concatenate(
                [W_k.T[0:128], W_k.T[128:256]], axis=1).astype(np.float16)),
            "wv_l": np.ascontiguousarray(np.concatenate(
                [W_v.T[0:128], W_v.T[128:256]], axis=1).astype(np.float16)),
            "wd1_l": np.ascontiguousarray(wd1_l),
            "wd2_l": np.ascontiguousarray(W_d2.T.astype(np.float16)),
            "wg1_l": np.ascontiguousarray(W_g1.T.astype(np.float16)),
            "wg2_l": np.ascontiguousarray(W_g2.T.astype(np.float16)),
            "colv": np.ascontiguousarray(colv),
            "esel": np.ascontiguousarray(esel),
            "masks": np.ascontiguousarray(
                np.concatenate([masks[0], masks[1]], axis=1)),
            "ident16": ident16,
            "ident32": ident32,
        })
    return maps


def kernel(**inputs):
    if "nc" not in _CACHE:
        _CACHE["nc"] = _build()
    nc = _CACHE["nc"]
    maps = _host_prep(inputs)
    res = run_bass_kernel_spmd(nc, maps, core_ids=list(range(8)))
    out = np.empty((B, NQ, DIM), np.float32)
    for core in range(8):
        b, h = core // 2, core % 2
        out[b, h * NQC:(h + 1) * NQC, :] = res.results[core]["out"]
    return out
